# revision 1
# baseline (speedup 1.0000x reference)
"""Trainium2 Bass kernel for Mistral-style attention with an INVERTED band mask.

Reference semantics (S=2048, E=4096, H=32, KV=8, D=128, WINDOW=1024):
  q/k/v projections -> RoPE(q,k) -> GQA attention where positions with
  |i-j| < 1024 are masked OUT (attend only to far positions) -> softmax ->
  out projection.

Sharding (8 cores, tensor-parallel by GQA group):
  core c owns KV head c and Q heads 4c..4c+3. Column-parallel QKV,
  row-parallel O projection; the 8 fp16 partial outputs are summed on host.

On-device layout: everything transposed so matmuls contract on partitions.
  Host passes hidden^T, Wq^T/Wk^T/Wv^T slices, Wo^T slice, RoPE tables
  (transposed, sign-folded), and two 128x128 triangular masks for the
  blocks that straddle the |i-j|=1024 boundary.

Block sparsity: score block (bi,bj) [128x128] is computed only when
  |bi-bj| >= 8; blocks at exactly |bi-bj|=8 get a triangular mask.

Softmax: scores^T layout [sk, sq]; exp on ScalarE (no max subtraction --
  scores are O(10) so fp32 exp is safe); denominator via ones-vector
  matmul on TensorE; normalize with reciprocal + partition_broadcast.
"""

import math
from contextlib import ExitStack

import numpy as np
import ml_dtypes

import concourse.bass as bass
import concourse.mybir as mybir
import concourse.tile as tile
from concourse import bacc
from concourse.bass_utils import run_bass_kernel_spmd

P = 128
S = 2048
E = 4096
D = 128
HPC = 4          # q heads per core
NE = E // P      # 32 e-tiles
NSCH = 4         # s-chunks of 512
SCH = S // NSCH  # 512
NST = S // P     # 16 s-tiles
NEO = 8          # output e-chunks of 512
SCALE = 1.0 / math.sqrt(D)
F16 = mybir.dt.float16
F32 = mybir.dt.float32
BF16 = mybir.dt.bfloat16


def _allowed_tiles(c):
    """For s-chunk c (query blocks bi=4c..4c+3), list (bj, lo, hi, mask, mpos):
    key tile bj is needed for query sub-tiles [lo, hi) (chunk-relative);
    mask in {None,'low','up'} applied at chunk-relative position mpos."""
    out = []
    bis = range(4 * c, 4 * c + 4)
    for bj in range(NST):
        ok = [bi for bi in bis if abs(bi - bj) >= 8]
        if not ok:
            continue
        lo = min(ok) - 4 * c
        hi = max(ok) + 1 - 4 * c
        assert ok == list(range(lo + 4 * c, hi + 4 * c)), (c, bj, ok)
        mask, mpos = None, 0
        if bj - 8 in ok:
            mask, mpos = "low", bj - 8 - 4 * c
        elif bj + 8 in ok:
            mask, mpos = "up", bj + 8 - 4 * c
        out.append((bj, lo, hi, mask, mpos))
    return out


def build_nc(debug=False):
    nc = bacc.Bacc("TRN2", target_bir_lowering=False, debug=False)
    hidT = nc.dram_tensor("hidT", (E, S), F16, kind="ExternalInput")
    wqT = nc.dram_tensor("wqT", (E, HPC * D), F16, kind="ExternalInput")
    wkT = nc.dram_tensor("wkT", (E, D), F16, kind="ExternalInput")
    wvT = nc.dram_tensor("wvT", (E, D), F16, kind="ExternalInput")
    woT = nc.dram_tensor("woT", (HPC * D, E), F16, kind="ExternalInput")
    cosT = nc.dram_tensor("cosT", (D, S), F32, kind="ExternalInput")
    sinT = nc.dram_tensor("sinT", (D, S), F32, kind="ExternalInput")
    mlow = nc.dram_tensor("mlow", (P, P), BF16, kind="ExternalInput")
    mup = nc.dram_tensor("mup", (P, P), BF16, kind="ExternalInput")
    outd = nc.dram_tensor("out", (S, E), F16, kind="ExternalOutput")
    if debug:
        qTd = nc.dram_tensor("qTd", (P, HPC, S), F16, kind="ExternalOutput")
        kTd = nc.dram_tensor("kTd", (P, S), F16, kind="ExternalOutput")
        vd = nc.dram_tensor("vd", (P, NST, D), F16, kind="ExternalOutput")
        attnd = nc.dram_tensor("attnd", (P, HPC, S), F16, kind="ExternalOutput")

    with tile.TileContext(nc) as tc, ExitStack() as ctx:
        const = ctx.enter_context(tc.tile_pool(name="const", bufs=1))

        wqT_r = wqT.rearrange("(eo p) d -> p eo d", p=P)
        wkT_r = wkT.rearrange("(eo p) d -> p eo d", p=P)
        wvT_r = wvT.rearrange("(eo p) d -> p eo d", p=P)
        wq_t, wk_t, wv_t = [], [], []
        for e in range(NE):
            wq = const.tile([P, HPC * D], F16, name=f"wq{e}")
            nc.sync.dma_start(wq[:], wqT_r[:, e, :])
            wq_t.append(wq)
            wk = const.tile([P, D], F16, name=f"wk{e}")
            nc.sync.dma_start(wk[:], wkT_r[:, e, :])
            wk_t.append(wk)
            wv = const.tile([P, D], F16, name=f"wv{e}")
            nc.sync.dma_start(wv[:], wvT_r[:, e, :])
            wv_t.append(wv)
        woT_r = woT.rearrange("(ho p) e -> p ho e", p=P)
        wo_t = []
        for h in range(HPC):
            wo = const.tile([P, E], F16, name=f"wo{h}")
            nc.sync.dma_start(wo[:], woT_r[:, h, :])
            wo_t.append(wo)
        cos_sb = const.tile([P, S], F32)
        nc.sync.dma_start(cos_sb[:], cosT[:])
        sin_sb = const.tile([P, S], F32)
        nc.sync.dma_start(sin_sb[:], sinT[:])
        ml_sb = const.tile([P, P], BF16)
        nc.sync.dma_start(ml_sb[:], mlow[:])
        mu_sb = const.tile([P, P], BF16)
        nc.sync.dma_start(mu_sb[:], mup[:])
        ones_sb = const.tile([P, 1], F16)
        nc.gpsimd.memset(ones_sb[:], 1.0)

        qT_sb = const.tile([P, HPC, S], F16)     # Q^T per head [d, s]
        kT_sb = const.tile([P, S], F16)          # K^T [d, s]
        v_sb = const.tile([P, NST, D], F16)      # V [s-tile, d]
        attn_sb = const.tile([P, HPC, S], F16)   # attn_out^T per head [d, s]

        hidp = ctx.enter_context(tc.tile_pool(name="hid", bufs=4))
        rp = ctx.enter_context(tc.tile_pool(name="rope", bufs=2))

        def rope_drain(src_psum):
            raw = rp.tile([P, SCH], F32, tag="raw", bufs=5)
            nc.any.tensor_copy(raw[:], src_psum)
            return raw

        def rope_apply(raw, dst_ap, c):
            rot = rp.tile([P, SCH], F32, tag="rot", bufs=2)
            nc.sync.dma_start(rot[0:64, :], raw[64:128, :])
            nc.sync.dma_start(rot[64:128, :], raw[0:64, :])
            t1 = rp.tile([P, SCH], F32, tag="t1", bufs=2)
            nc.vector.tensor_tensor(
                t1[:], raw[:], cos_sb[:, c * SCH:(c + 1) * SCH], mybir.AluOpType.mult)
            t2 = rp.tile([P, SCH], F32, tag="t2", bufs=2)
            nc.vector.tensor_tensor(
                t2[:], rot[:], sin_sb[:, c * SCH:(c + 1) * SCH], mybir.AluOpType.mult)
            nc.vector.tensor_tensor(dst_ap, t1[:], t2[:], mybir.AluOpType.add)

        # ---- Phase 1: QKV projections (+RoPE) ----
        with tc.tile_pool(name="p1psum", bufs=1, space="PSUM") as p1, \
             tc.tile_pool(name="p1kv", bufs=2, space="PSUM") as p1kv:
            for c in range(NSCH):
                psq = p1.tile([P, HPC, SCH], F32, tag="psq")   # 4 banks
                psk = p1kv.tile([P, SCH], F32, tag="psk")      # 2 banks
                psvT = p1kv.tile([P, SCH], F32, tag="psv")     # 2 banks (V^T)
                for e in range(NE):
                    ht = hidp.tile([P, SCH], F16, tag="hid")
                    nc.sync.dma_start(
                        ht[:], hidT[e * P:(e + 1) * P, c * SCH:(c + 1) * SCH])
                    st = (e == 0)
                    sp = (e == NE - 1)
                    for h in range(HPC):
                        nc.tensor.matmul(
                            psq[:, h, :], wq_t[e][:, h * D:(h + 1) * D], ht[:],
                            start=st, stop=sp)
                    nc.tensor.matmul(psk[:], wk_t[e][:], ht[:], start=st, stop=sp)
                    nc.tensor.matmul(psvT[:], wv_t[e][:], ht[:], start=st, stop=sp)
                vstage = rp.tile([P, SCH], F16, tag="vstage", bufs=2)
                nc.any.tensor_copy(vstage[:], psvT[:])
                nc.sync.dma_start_transpose(
                    v_sb[:, c * 4:(c + 1) * 4, :], vstage[:])
                kraw = rope_drain(psk[:])
                qraws = [rope_drain(psq[:, h, :]) for h in range(HPC)]
                rope_apply(kraw, kT_sb[:, c * SCH:(c + 1) * SCH], c)
                for h in range(HPC):
                    rope_apply(qraws[h], qT_sb[:, h, c * SCH:(c + 1) * SCH], c)

        # ---- Phase 2+3: attention interleaved with O projection ----
        ep = ctx.enter_context(tc.tile_pool(name="expp", bufs=3))
        np_pool = ctx.enter_context(tc.tile_pool(name="normp", bufs=2))
        osp = ctx.enter_context(tc.tile_pool(name="ostage", bufs=4))
        with tc.tile_pool(name="apsum", bufs=2, space="PSUM") as ap:
            for c in range(NSCH):
                blocks = _allowed_tiles(c)
                for h in range(HPC):
                    psa = ap.tile([P, SCH], F32, tag="psa")
                    psd = ap.tile([1, SCH], F32, tag="psd")
                    nblk = len(blocks)
                    for idx, (bj, lo, hi, mask, mpos) in enumerate(blocks):
                        n = (hi - lo) * P
                        pss = ap.tile([P, SCH], F32, tag="pss")
                        nc.tensor.matmul(
                            pss[:, :n],
                            kT_sb[:, bj * P:(bj + 1) * P],
                            qT_sb[:, h, c * SCH + lo * P: c * SCH + hi * P],
                            start=True, stop=True)
                        et = ep.tile([P, SCH], BF16, tag="exp")
                        if n < SCH:
                            nc.any.memzero(et[:])
                        nc.scalar.activation(
                            et[:, lo * P:hi * P], pss[:, :n],
                            mybir.ActivationFunctionType.Exp, scale=SCALE)
                        if mask == "low":
                            nc.vector.tensor_tensor(
                                et[:, mpos * P:(mpos + 1) * P],
                                et[:, mpos * P:(mpos + 1) * P],
                                ml_sb[:], mybir.AluOpType.mult)
                        elif mask == "up":
                            nc.vector.tensor_tensor(
                                et[:, mpos * P:(mpos + 1) * P],
                                et[:, mpos * P:(mpos + 1) * P],
                                mu_sb[:], mybir.AluOpType.mult)
                        nc.tensor.matmul(
                            psa[:], v_sb[:, bj, :], et[:],
                            start=(idx == 0), stop=(idx == nblk - 1))
                        nc.tensor.matmul(
                            psd[:], ones_sb[:], et[:],
                            start=(idx == 0), stop=(idx == nblk - 1))
                    rc = np_pool.tile([1, SCH], F32, tag="recip")
                    nc.vector.reciprocal(rc[:], psd[:])
                    bc = np_pool.tile([P, SCH], F32, tag="bcast")
                    nc.gpsimd.partition_broadcast(bc[:], rc[:])
                    nc.vector.tensor_tensor(
                        attn_sb[:, h, c * SCH:(c + 1) * SCH], psa[:], bc[:],
                        mybir.AluOpType.mult)
                # O projection for this chunk's four s-tiles (overlaps next
                # chunk's attention on PE via shared pool slots)
                for st in range(4 * c, 4 * c + 4):
                    orow = osp.tile([P, E], F16, tag="orow", bufs=2)
                    for eo in range(NEO):
                        pso = ap.tile([P, SCH], F32, tag="pso")
                        for h in range(HPC):
                            nc.tensor.matmul(
                                pso[:],
                                attn_sb[:, h, st * P:(st + 1) * P],
                                wo_t[h][:, eo * SCH:(eo + 1) * SCH],
                                start=(h == 0), stop=(h == HPC - 1))
                        nc.any.tensor_copy(
                            orow[:, eo * SCH:(eo + 1) * SCH], pso[:])
                    nc.sync.dma_start(outd[st * P:(st + 1) * P, :], orow[:])
        if debug:
            nc.sync.dma_start(qTd[:], qT_sb[:])
            nc.sync.dma_start(kTd[:], kT_sb[:])
            nc.sync.dma_start(vd[:], v_sb[:])
            nc.sync.dma_start(attnd[:], attn_sb[:])
    nc.compile()
    return nc


_NC_CACHE = {}


def get_nc():
    if "nc" not in _NC_CACHE:
        _NC_CACHE["nc"] = build_nc()
    return _NC_CACHE["nc"]


def make_in_maps(hidden_states, Wq, Wk, Wv, Wo):
    hid = np.asarray(hidden_states).reshape(S, E)
    hidT16 = np.ascontiguousarray(hid.T).astype(np.float16)

    inv = 1.0 / (10000.0 ** (np.arange(0, D, 2, dtype=np.float64) / D))
    t = np.arange(S, dtype=np.float64)
    fr = np.outer(t, inv)                      # [S, 64]
    emb = np.concatenate([fr, fr], axis=1)     # [S, 128]
    cosT = np.ascontiguousarray(np.cos(emb).T).astype(np.float32)
    sinT = np.ascontiguousarray(np.sin(emb).T).astype(np.float32)
    sinT[:64] *= -1.0                          # rotate_half sign fold

    jj = np.arange(P)[:, None]
    ii = np.arange(P)[None, :]
    mlow = (jj >= ii).astype(ml_dtypes.bfloat16)   # block bj-bi=8: j-i>=1024
    mup = (ii >= jj).astype(ml_dtypes.bfloat16)    # block bi-bj=8: i-j>=1024

    in_maps = []
    for c in range(8):
        qsl = slice(c * 512, (c + 1) * 512)
        ksl = slice(c * 128, (c + 1) * 128)
        in_maps.append({
            "hidT": hidT16,
            "wqT": np.ascontiguousarray(Wq[qsl].T).astype(np.float16),
            "wkT": np.ascontiguousarray(Wk[ksl].T).astype(np.float16),
            "wvT": np.ascontiguousarray(Wv[ksl].T).astype(np.float16),
            "woT": np.ascontiguousarray(Wo[:, qsl].T).astype(np.float16),
            "cosT": cosT,
            "sinT": sinT,
            "mlow": mlow,
            "mup": mup,
        })
    return in_maps


def run(in_maps, **kwargs):
    nc = get_nc()
    return run_bass_kernel_spmd(nc, in_maps, core_ids=list(range(8)), **kwargs)


def kernel(hidden_states, Wq, Wk, Wv, Wo):
    in_maps = make_in_maps(hidden_states, Wq, Wk, Wv, Wo)
    res = run(in_maps)
    out = np.zeros((S, E), dtype=np.float32)
    for r in res.results:
        out += r["out"].astype(np.float32)
    return out.reshape(1, S, E)



# revision 6
# speedup vs baseline: 1.2152x; 1.2152x over previous
"""Trainium2 Bass kernel for Mistral-style attention with an INVERTED band mask.

Reference semantics (S=2048, E=4096, H=32, KV=8, D=128, WINDOW=1024):
  q/k/v projections -> RoPE(q,k) -> GQA attention where positions with
  |i-j| < 1024 are masked OUT (attend only to far positions) -> softmax ->
  out projection.

Sharding (8 cores, tensor-parallel by GQA group):
  core c owns KV head c and Q heads 4c..4c+3. Column-parallel QKV,
  row-parallel O projection; the 8 fp16 partial outputs are summed on host.

On-device layout: everything transposed so matmuls contract on partitions.
  Host passes hidden^T, Wq^T/Wk^T/Wv^T slices, Wo^T slice, RoPE tables
  (transposed, sign-folded; sin2 = sin rolled by 64 so the rotate-half can
  happen AFTER the multiply via one SBUF->SBUF DMA swap), and two 128x128
  triangular masks for the blocks straddling the |i-j|=1024 boundary.

Block sparsity: score block (bi,bj) [128x128] is computed only when
  |bi-bj| >= 8; blocks at exactly |bi-bj|=8 get a triangular mask.

Scheduling notes (v2):
  - Phase-1 chunk order [0,1,3,2] and attention order [3,2,0,1] so the
    first attention chunk's inputs (K/V of s<1024, Q of chunk 3) are ready
    before phase 1 finishes -> no PE gap at the transition.
  - O-projection matmuls of the previous attention chunk are interleaved
    into the next chunk's block loop as PE filler while ScalarE runs exp.
  - Scores are software-pipelined one block ahead (pss bufs=3) so the PE
    never sits behind exp in its own FIFO.
  - AV / denominator matmuls restricted to the valid query range per block.
  - Copies pinned to DVE (nc.any would put them on ScalarE, starving exp);
    softmax reciprocal via reciprocal_approx_fast (~5x faster).
"""

import math
from contextlib import ExitStack

import numpy as np
import ml_dtypes

import concourse.bass as bass
import concourse.mybir as mybir
import concourse.tile as tile
from concourse import bacc
from concourse.bass_utils import run_bass_kernel_spmd

P = 128
S = 2048
E = 4096
D = 128
HPC = 4          # q heads per core
NE = E // P      # 32 e-tiles
NSCH = 4         # s-chunks of 512
SCH = S // NSCH  # 512
NST = S // P     # 16 s-tiles
NEO = 8          # output e-chunks of 512
SCALE = 1.0 / math.sqrt(D)
F16 = mybir.dt.float16
F32 = mybir.dt.float32
BF16 = mybir.dt.bfloat16

P1_ORDER = [0, 1, 3, 2]   # phase-1 chunk order
AT_ORDER = [3, 2, 0, 1]   # attention chunk order


def _allowed_tiles(c):
    """For s-chunk c (query blocks bi=4c..4c+3), list (bj, lo, hi, mask, mpos):
    key tile bj is needed for query sub-tiles [lo, hi) (chunk-relative);
    mask in {None,'low','up'} applied at chunk-relative position mpos."""
    out = []
    bis = range(4 * c, 4 * c + 4)
    for bj in range(NST):
        ok = [bi for bi in bis if abs(bi - bj) >= 8]
        if not ok:
            continue
        lo = min(ok) - 4 * c
        hi = max(ok) + 1 - 4 * c
        assert ok == list(range(lo + 4 * c, hi + 4 * c)), (c, bj, ok)
        mask, mpos = None, 0
        if bj - 8 in ok:
            mask, mpos = "low", bj - 8 - 4 * c
        elif bj + 8 in ok:
            mask, mpos = "up", bj + 8 - 4 * c
        out.append((bj, lo, hi, mask, mpos))
    return out


def build_nc(debug=False):
    nc = bacc.Bacc("TRN2", target_bir_lowering=False, debug=False)
    hidT = nc.dram_tensor("hidT", (E, S), F16, kind="ExternalInput")
    wqT = nc.dram_tensor("wqT", (E, HPC * D), F16, kind="ExternalInput")
    wkT = nc.dram_tensor("wkT", (E, D), F16, kind="ExternalInput")
    wvT = nc.dram_tensor("wvT", (E, D), F16, kind="ExternalInput")
    woT = nc.dram_tensor("woT", (HPC * D, E), F16, kind="ExternalInput")
    cosT = nc.dram_tensor("cosT", (D, S), F32, kind="ExternalInput")
    sin2T = nc.dram_tensor("sin2T", (D, S), F32, kind="ExternalInput")
    mlow = nc.dram_tensor("mlow", (P, P), BF16, kind="ExternalInput")
    mup = nc.dram_tensor("mup", (P, P), BF16, kind="ExternalInput")
    outd = nc.dram_tensor("out", (S, E), F16, kind="ExternalOutput")

    with tile.TileContext(nc) as tc, ExitStack() as ctx:
        const = ctx.enter_context(tc.tile_pool(name="const", bufs=1))

        wqT_r = wqT.rearrange("(eo p) d -> p eo d", p=P)
        wkT_r = wkT.rearrange("(eo p) d -> p eo d", p=P)
        wvT_r = wvT.rearrange("(eo p) d -> p eo d", p=P)
        hidT_r = hidT.rearrange("(eo p) s -> p eo s", p=P)

        # SBUF homes (persistent)
        qT_sb = const.tile([P, HPC, S], F16)     # Q^T per head [d, s]
        kT_sb = const.tile([P, S], F16)          # K^T [d, s]
        v_sb = const.tile([P, NST, D], F16)      # V [s-tile, d]
        attn_sb = const.tile([P, HPC, S], F16)   # attn_out^T per head [d, s]

        wq_t = [const.tile([P, HPC * D], F16, name=f"wq{e}") for e in range(NE)]
        wk_t = [const.tile([P, D], F16, name=f"wk{e}") for e in range(NE)]
        wv_t = [const.tile([P, D], F16, name=f"wv{e}") for e in range(NE)]
        wo_t = [const.tile([P, E], F16, name=f"wo{h}") for h in range(HPC)]
        cos_sb = const.tile([P, S], F32)
        sin2_sb = const.tile([P, S], F32)
        ml_sb = const.tile([P, P], BF16)
        mu_sb = const.tile([P, P], BF16)
        ones_sb = const.tile([P, 1], F16)

        hidp = ctx.enter_context(tc.tile_pool(name="hid", bufs=6))
        hid_tiles = {}

        def issue_hid_dma(c, e):
            ht = hidp.tile([P, SCH], F16, tag="hid")
            nc.sync.dma_start(ht[:], hidT_r[:, e, c * SCH:(c + 1) * SCH])
            hid_tiles[(c, e)] = ht

        # --- DMA program: JIT-ordered so the first matmul's inputs land
        # first; remaining weights stream in behind the e-loop. ---
        c0 = P1_ORDER[0]
        for e in range(NE):
            nc.sync.dma_start(wq_t[e][:], wqT_r[:, e, :])
            nc.sync.dma_start(wk_t[e][:], wkT_r[:, e, :])
            nc.sync.dma_start(wv_t[e][:], wvT_r[:, e, :])
            issue_hid_dma(c0, e)
            if e == 0:
                nc.gpsimd.memset(ones_sb[:], 1.0)
            if e == 4:
                nc.sync.dma_start(cos_sb[:], cosT[:])
                nc.sync.dma_start(sin2_sb[:], sin2T[:])
            if e == 8:
                nc.sync.dma_start(ml_sb[:], mlow[:])
                nc.sync.dma_start(mu_sb[:], mup[:])
            if e == 12:
                for h in range(HPC):
                    nc.sync.dma_start(
                        wo_t[h][:], woT.rearrange("(ho p) e -> p ho e", p=P)[:, h, :])

        rp = ctx.enter_context(tc.tile_pool(name="rope", bufs=2))

        def rope_apply(src_psum, dst_ap, c):
            """dst = src*cos + rot64(src*sin2); reads src from PSUM twice
            (no raw drain); the rotate-half is a SBUF<->SBUF DMA swap."""
            csl = slice(c * SCH, (c + 1) * SCH)
            t1 = rp.tile([P, SCH], F32, tag="t1", bufs=3)
            nc.vector.tensor_tensor(t1[:], src_psum, cos_sb[:, csl],
                                    mybir.AluOpType.mult)
            t2 = rp.tile([P, SCH], F32, tag="t2", bufs=3)
            nc.vector.tensor_tensor(t2[:], src_psum, sin2_sb[:, csl],
                                    mybir.AluOpType.mult)
            rot = rp.tile([P, SCH], F32, tag="rot", bufs=3)
            nc.sync.dma_start(rot[0:64, :], t2[64:128, :])
            nc.sync.dma_start(rot[64:128, :], t2[0:64, :])
            nc.vector.tensor_tensor(dst_ap, t1[:], rot[:], mybir.AluOpType.add)

        # ---- Phase 1: QKV projections (+RoPE) ----
        with tc.tile_pool(name="p1q", bufs=5, space="PSUM") as p1q, \
             tc.tile_pool(name="p1k", bufs=1, space="PSUM") as p1k, \
             tc.tile_pool(name="p1v", bufs=2, space="PSUM") as p1v:
            for ci, c in enumerate(P1_ORDER):
                psq = [p1q.tile([P, SCH], F32, tag="psq", name=f"psq{h}")
                       for h in range(HPC)]
                psk = p1k.tile([P, SCH], F32, tag="psk")
                psvT = p1v.tile([P, SCH], F32, tag="psv")
                for e in range(NE):
                    ht = hid_tiles.pop((c, e))
                    st = (e == 0)
                    sp = (e == NE - 1)
                    nc.tensor.matmul(psk[:], wk_t[e][:], ht[:], start=st, stop=sp)
                    nc.tensor.matmul(psvT[:], wv_t[e][:], ht[:], start=st, stop=sp)
                    for h in range(HPC):
                        nc.tensor.matmul(
                            psq[h][:], wq_t[e][:, h * D:(h + 1) * D], ht[:],
                            start=st, stop=sp)
                    # prefetch next chunk's hid tile
                    if ci + 1 < NSCH:
                        issue_hid_dma(P1_ORDER[ci + 1], e)
                vstage = rp.tile([P, SCH], F16, tag="vstage", bufs=2)
                nc.vector.tensor_copy(vstage[:], psvT[:])
                nc.sync.dma_start_transpose(
                    v_sb[:, c * 4:(c + 1) * 4, :], vstage[:])
                rope_apply(psk[:], kT_sb[:, c * SCH:(c + 1) * SCH], c)
                for h in range(HPC):
                    rope_apply(psq[h][:], qT_sb[:, h, c * SCH:(c + 1) * SCH], c)

        # ---- Phase 2+3: attention with O-projection interleaved as PE filler ----
        ep = ctx.enter_context(tc.tile_pool(name="expp", bufs=3))
        np_pool = ctx.enter_context(tc.tile_pool(name="normp", bufs=2))
        osp = ctx.enter_context(tc.tile_pool(name="ostage", bufs=2))

        with tc.tile_pool(name="apss", bufs=3, space="PSUM") as pss_pool, \
             tc.tile_pool(name="apsa", bufs=2, space="PSUM") as psa_pool, \
             tc.tile_pool(name="apsd", bufs=1, space="PSUM") as psd_pool, \
             tc.tile_pool(name="apso", bufs=2, space="PSUM") as pso_pool:

            orows = {}      # st -> staged output row awaiting DMA
            fillers = []    # pending O-proj (st, eo) units for PE filler

            def emit_filler(n):
                for _ in range(n):
                    if not fillers:
                        return
                    st, eo = fillers.pop(0)
                    pso = pso_pool.tile([P, SCH], F32, tag="pso")
                    for h in range(HPC):
                        nc.tensor.matmul(
                            pso[:],
                            attn_sb[:, h, st * P:(st + 1) * P],
                            wo_t[h][:, eo * SCH:(eo + 1) * SCH],
                            start=(h == 0), stop=(h == HPC - 1))
                    orow = orows[st]
                    nc.vector.tensor_copy(orow[:, eo * SCH:(eo + 1) * SCH], pso[:])
                    if eo == NEO - 1:
                        nc.sync.dma_start(outd[st * P:(st + 1) * P, :], orow[:])
                        del orows[st]

            def queue_oproj(c):
                for st in range(4 * c, 4 * c + 4):
                    orows[st] = osp.tile([P, E], F16, tag="orow", name=f"orow{st}")
                    for eo in range(NEO):
                        fillers.append((st, eo))

            for ai, c in enumerate(AT_ORDER):
                blocks = _allowed_tiles(c)
                nblk = len(blocks)
                for h in range(HPC):
                    psa = psa_pool.tile([P, SCH], F32, tag="psa")
                    psd = psd_pool.tile([1, SCH], F32, tag="psd")
                    # software pipeline: scores one block ahead of exp/AV
                    pend = None  # (idx, bj, lo, hi, pss, et)
                    for idx, (bj, lo, hi, mask, mpos) in enumerate(blocks):
                        n = (hi - lo) * P
                        pss = pss_pool.tile([P, SCH], F32, tag="pss")
                        nc.tensor.matmul(
                            pss[:, :n],
                            kT_sb[:, bj * P:(bj + 1) * P],
                            qT_sb[:, h, c * SCH + lo * P: c * SCH + hi * P],
                            start=True, stop=True)
                        et = ep.tile([P, SCH], BF16, tag="exp")
                        nc.scalar.activation(
                            et[:, lo * P:hi * P], pss[:, :n],
                            mybir.ActivationFunctionType.Exp, scale=SCALE)
                        if mask == "low":
                            nc.vector.tensor_tensor(
                                et[:, mpos * P:(mpos + 1) * P],
                                et[:, mpos * P:(mpos + 1) * P],
                                ml_sb[:], mybir.AluOpType.mult)
                        elif mask == "up":
                            nc.vector.tensor_tensor(
                                et[:, mpos * P:(mpos + 1) * P],
                                et[:, mpos * P:(mpos + 1) * P],
                                mu_sb[:], mybir.AluOpType.mult)
                        if pend is not None:
                            _emit_av(nc, psa, psd, v_sb, ones_sb, pend, nblk)
                            emit_filler(1)
                        pend = (idx, bj, lo, hi, et)
                    _emit_av(nc, psa, psd, v_sb, ones_sb, pend, nblk)
                    emit_filler(1)
                    # normalize: rc = ~1/denom, broadcast, multiply
                    rc = np_pool.tile([1, SCH], F32, tag="recip")
                    nc.vector.reciprocal_approx_fast(rc[:], psd[:])
                    bc = np_pool.tile([P, SCH], F32, tag="bcast")
                    nc.gpsimd.partition_broadcast(bc[:], rc[:])
                    nc.vector.tensor_tensor(
                        attn_sb[:, h, c * SCH:(c + 1) * SCH], psa[:], bc[:],
                        mybir.AluOpType.mult)
                    emit_filler(2)
                if ai > 0:
                    emit_filler(len(fillers))  # drain any leftovers
                queue_oproj(c)
            emit_filler(len(fillers))
    nc.compile()
    return nc


def _emit_av(nc, psa, psd, v_sb, ones_sb, pend, nblk):
    idx, bj, lo, hi, et = pend
    sl = slice(lo * P, hi * P)
    nc.tensor.matmul(
        psa[:, sl], v_sb[:, bj, :], et[:, sl],
        start=(idx == 0), stop=(idx == nblk - 1))
    nc.tensor.matmul(
        psd[:, sl], ones_sb[:], et[:, sl],
        start=(idx == 0), stop=(idx == nblk - 1))


_NC_CACHE = {}


def get_nc():
    if "nc" not in _NC_CACHE:
        _NC_CACHE["nc"] = build_nc()
    return _NC_CACHE["nc"]


def make_in_maps(hidden_states, Wq, Wk, Wv, Wo):
    hid = np.asarray(hidden_states).reshape(S, E)
    hidT16 = np.ascontiguousarray(hid.T).astype(np.float16)

    inv = 1.0 / (10000.0 ** (np.arange(0, D, 2, dtype=np.float64) / D))
    t = np.arange(S, dtype=np.float64)
    fr = np.outer(t, inv)                      # [S, 64]
    emb = np.concatenate([fr, fr], axis=1)     # [S, 128]
    cosT = np.ascontiguousarray(np.cos(emb).T).astype(np.float32)
    sinF = np.ascontiguousarray(np.sin(emb).T).astype(np.float32)
    sinF[:64] *= -1.0                          # rotate_half sign fold
    sin2T = np.ascontiguousarray(np.roll(sinF, -64, axis=0))

    jj = np.arange(P)[:, None]
    ii = np.arange(P)[None, :]
    mlow = (jj >= ii).astype(ml_dtypes.bfloat16)   # block bj-bi=8: j-i>=1024
    mup = (ii >= jj).astype(ml_dtypes.bfloat16)    # block bi-bj=8: i-j>=1024

    in_maps = []
    for c in range(8):
        qsl = slice(c * 512, (c + 1) * 512)
        ksl = slice(c * 128, (c + 1) * 128)
        in_maps.append({
            "hidT": hidT16,
            "wqT": np.ascontiguousarray(Wq[qsl].T).astype(np.float16),
            "wkT": np.ascontiguousarray(Wk[ksl].T).astype(np.float16),
            "wvT": np.ascontiguousarray(Wv[ksl].T).astype(np.float16),
            "woT": np.ascontiguousarray(Wo[:, qsl].T).astype(np.float16),
            "cosT": cosT,
            "sin2T": sin2T,
            "mlow": mlow,
            "mup": mup,
        })
    return in_maps


def run(in_maps, **kwargs):
    nc = get_nc()
    return run_bass_kernel_spmd(nc, in_maps, core_ids=list(range(8)), **kwargs)


def kernel(hidden_states, Wq, Wk, Wv, Wo):
    in_maps = make_in_maps(hidden_states, Wq, Wk, Wv, Wo)
    res = run(in_maps)
    out = np.zeros((S, E), dtype=np.float32)
    for r in res.results:
        out += r["out"].astype(np.float32)
    return out.reshape(1, S, E)


# revision 7
# speedup vs baseline: 1.3607x; 1.1198x over previous
"""Trainium2 Bass kernel for Mistral-style attention with an INVERTED band mask.

Reference semantics (S=2048, E=4096, H=32, KV=8, D=128, WINDOW=1024):
  q/k/v projections -> RoPE(q,k) -> GQA attention where positions with
  |i-j| < 1024 are masked OUT (attend only to far positions) -> softmax ->
  out projection.

Sharding (8 cores, tensor-parallel by GQA group):
  core c owns KV head c and Q heads 4c..4c+3. Column-parallel QKV,
  row-parallel O projection; the 8 fp16 partial outputs are summed on host.

On-device layout: everything transposed so matmuls contract on partitions.
  Host passes hidden^T, fused Wqkv^T slice, Wo^T slice, RoPE tables
  (transposed, sign-folded; sin2 = sin rolled by 64 so the rotate-half can
  happen AFTER the multiply via one SBUF->SBUF DMA swap), and two 128x128
  triangular masks for the blocks straddling the |i-j|=1024 boundary.

Block sparsity: score block (bi,bj) [128x128] is computed only when
  |bi-bj| >= 8; blocks at exactly |bi-bj|=8 get a triangular mask.

Scheduling notes (v3):
  - Two HWDGE rings: Sync carries latency-critical streams (hid tiles JIT
    slot-gated, output rows); Scalar carries bulk weights + rope swaps +
    V transposes, so a slot-wait on one stream can't head-of-line block
    the other.
  - Phase-1 chunk order [0,1,3,2] and attention order [3,2,0,1] so the
    first attention chunk's inputs are ready before phase 1 finishes.
  - O-projection matmuls of the previous attention chunk interleave into
    the block loop as PE filler while ScalarE runs exp; scores pipelined
    one block ahead (pss bufs=3); psa/pso share one 4-buf PSUM tag.
  - AV / denominator matmuls restricted to the valid query range; psd
    drained by a ScalarE copy so its bank frees independent of the DVE
    queue; reciprocal via reciprocal_approx_fast.
"""

import math
from contextlib import ExitStack

import numpy as np
import ml_dtypes

import concourse.bass as bass
import concourse.mybir as mybir
import concourse.tile as tile
from concourse import bacc
from concourse.bass_utils import run_bass_kernel_spmd

P = 128
S = 2048
E = 4096
D = 128
HPC = 4          # q heads per core
NE = E // P      # 32 e-tiles
NE2 = NE // 2    # 16 double-e tiles
NSCH = 4         # s-chunks of 512
SCH = S // NSCH  # 512
NST = S // P     # 16 s-tiles
NEO = 8          # output e-chunks of 512
WQKV = HPC * D + 2 * D  # 768 fused qkv weight cols per e-tile
SCALE = 1.0 / math.sqrt(D)
F16 = mybir.dt.float16
F32 = mybir.dt.float32
BF16 = mybir.dt.bfloat16

P1_ORDER = [0, 1, 3, 2]   # phase-1 chunk order
AT_ORDER = [3, 2, 0, 1]   # attention chunk order
HID_AHEAD = 6             # hid DMA lookahead (in [128,2,512] tiles)


def _allowed_tiles(c):
    """For s-chunk c (query blocks bi=4c..4c+3), list (bj, lo, hi, mask, mpos):
    key tile bj is needed for query sub-tiles [lo, hi) (chunk-relative);
    mask in {None,'low','up'} applied at chunk-relative position mpos."""
    out = []
    bis = range(4 * c, 4 * c + 4)
    for bj in range(NST):
        ok = [bi for bi in bis if abs(bi - bj) >= 8]
        if not ok:
            continue
        lo = min(ok) - 4 * c
        hi = max(ok) + 1 - 4 * c
        assert ok == list(range(lo + 4 * c, hi + 4 * c)), (c, bj, ok)
        mask, mpos = None, 0
        if bj - 8 in ok:
            mask, mpos = "low", bj - 8 - 4 * c
        elif bj + 8 in ok:
            mask, mpos = "up", bj + 8 - 4 * c
        out.append((bj, lo, hi, mask, mpos))
    return out


def build_nc(debug=False):
    nc = bacc.Bacc("TRN2", target_bir_lowering=False, debug=False)
    hidT = nc.dram_tensor("hidT", (E, S), F16, kind="ExternalInput")
    wqkvT = nc.dram_tensor("wqkvT", (E, WQKV), F16, kind="ExternalInput")
    woT = nc.dram_tensor("woT", (HPC * D, E), F16, kind="ExternalInput")
    cosT = nc.dram_tensor("cosT", (D, S), F32, kind="ExternalInput")
    sin2T = nc.dram_tensor("sin2T", (D, S), F32, kind="ExternalInput")
    mlow = nc.dram_tensor("mlow", (P, P), BF16, kind="ExternalInput")
    mup = nc.dram_tensor("mup", (P, P), BF16, kind="ExternalInput")
    outd = nc.dram_tensor("out", (S, E), F16, kind="ExternalOutput")

    with tile.TileContext(nc) as tc, ExitStack() as ctx:
        const = ctx.enter_context(tc.tile_pool(name="const", bufs=1))

        wqkvT_r = wqkvT.rearrange("(eo p) d -> p eo d", p=P)
        woT_r = woT.rearrange("(ho p) e -> p ho e", p=P)
        hidT_r = hidT.rearrange("(eo p) s -> p eo s", p=P)

        # SBUF homes (persistent)
        qT_sb = const.tile([P, HPC, S], F16)     # Q^T per head [d, s]
        kT_sb = const.tile([P, S], F16)          # K^T [d, s]
        v_sb = const.tile([P, NST, D], F16)      # V [s-tile, d]
        attn_sb = const.tile([P, HPC, S], F16)   # attn_out^T per head [d, s]

        wqkv_t = [const.tile([P, WQKV], F16, name=f"wqkv{e}") for e in range(NE)]
        wo_t = [const.tile([P, E], F16, name=f"wo{h}") for h in range(HPC)]
        cos_sb = const.tile([P, S], F32)
        sin2_sb = const.tile([P, S], F32)
        ml_sb = const.tile([P, P], BF16)
        mu_sb = const.tile([P, P], BF16)
        ones_sb = const.tile([P, 1], F16)

        def wq_ap(e, h):
            return wqkv_t[e][:, h * D:(h + 1) * D]

        def wk_ap(e):
            return wqkv_t[e][:, HPC * D:HPC * D + D]

        def wv_ap(e):
            return wqkv_t[e][:, HPC * D + D:]

        hidp = ctx.enter_context(tc.tile_pool(name="hid", bufs=HID_AHEAD))
        hid_tiles = {}

        def issue_hid_dma(c, e2):
            ht = hidp.tile([P, 2, SCH], F16, tag="hid")
            nc.sync.dma_start(
                ht[:], hidT_r[:, 2 * e2:2 * e2 + 2, c * SCH:(c + 1) * SCH])
            hid_tiles[(c, e2)] = ht

        # --- upfront DMA program (scalar ring for weights; sync for hid) ---
        c0 = P1_ORDER[0]
        for j in range(HID_AHEAD):
            issue_hid_dma(c0, j)
            nc.scalar.dma_start(wqkv_t[2 * j][:], wqkvT_r[:, 2 * j, :])
            nc.scalar.dma_start(wqkv_t[2 * j + 1][:], wqkvT_r[:, 2 * j + 1, :])
            if j == 0:
                nc.gpsimd.memset(ones_sb[:], 1.0)
            if j == 2:
                nc.scalar.dma_start(cos_sb[:], cosT[:])
                nc.scalar.dma_start(sin2_sb[:], sin2T[:])
            if j == 4:
                nc.scalar.dma_start(ml_sb[:], mlow[:])
                nc.scalar.dma_start(mu_sb[:], mup[:])
        for e in range(2 * HID_AHEAD, NE):
            nc.scalar.dma_start(wqkv_t[e][:], wqkvT_r[:, e, :])

        rp = ctx.enter_context(tc.tile_pool(name="rope", bufs=2))

        def rope_apply(src_psum, dst_ap, c):
            """dst = src*cos + rot64(src*sin2); reads src from PSUM twice
            (no raw drain); the rotate-half is a SBUF<->SBUF DMA swap on
            the scalar ring."""
            csl = slice(c * SCH, (c + 1) * SCH)
            t1 = rp.tile([P, SCH], F32, tag="t1", bufs=2)
            nc.vector.tensor_tensor(t1[:], src_psum, cos_sb[:, csl],
                                    mybir.AluOpType.mult)
            t2 = rp.tile([P, SCH], F32, tag="t2", bufs=2)
            nc.vector.tensor_tensor(t2[:], src_psum, sin2_sb[:, csl],
                                    mybir.AluOpType.mult)
            rot = rp.tile([P, SCH], F32, tag="rot", bufs=2)
            nc.scalar.dma_start(rot[0:64, :], t2[64:128, :])
            nc.scalar.dma_start(rot[64:128, :], t2[0:64, :])
            nc.vector.tensor_tensor(dst_ap, t1[:], rot[:], mybir.AluOpType.add)

        # ---- Phase 1: QKV projections (+RoPE) ----
        with tc.tile_pool(name="p1q", bufs=6, space="PSUM") as p1q, \
             tc.tile_pool(name="p1k", bufs=1, space="PSUM") as p1k, \
             tc.tile_pool(name="p1v", bufs=1, space="PSUM") as p1v:
            for ci, c in enumerate(P1_ORDER):
                psq = [p1q.tile([P, SCH], F32, tag="psq", name=f"psq{h}")
                       for h in range(HPC)]
                psk = p1k.tile([P, SCH], F32, tag="psk")
                psvT = p1v.tile([P, SCH], F32, tag="psv")
                for e2 in range(NE2):
                    ht = hid_tiles.pop((c, e2))
                    for j in range(2):
                        e = 2 * e2 + j
                        hap = ht[:, j, :]
                        st = (e == 0)
                        sp = (e == NE - 1)
                        nc.tensor.matmul(psk[:], wk_ap(e), hap, start=st, stop=sp)
                        nc.tensor.matmul(psvT[:], wv_ap(e), hap, start=st, stop=sp)
                        for h in range(HPC):
                            nc.tensor.matmul(psq[h][:], wq_ap(e, h), hap,
                                             start=st, stop=sp)
                    # JIT prefetch with HID_AHEAD tiles of lookahead
                    nxt = e2 + HID_AHEAD
                    if nxt < NE2:
                        issue_hid_dma(c, nxt)
                    elif ci + 1 < NSCH:
                        issue_hid_dma(P1_ORDER[ci + 1], nxt - NE2)
                vstage = rp.tile([P, SCH], F16, tag="vstage", bufs=2)
                nc.vector.tensor_copy(vstage[:], psvT[:])
                nc.scalar.dma_start_transpose(
                    v_sb[:, c * 4:(c + 1) * 4, :], vstage[:])
                rope_apply(psk[:], kT_sb[:, c * SCH:(c + 1) * SCH], c)
                for h in range(HPC):
                    rope_apply(psq[h][:], qT_sb[:, h, c * SCH:(c + 1) * SCH], c)
                if ci == 1:
                    # bulk wo loads: needed first ~40us into attention
                    for h in range(HPC):
                        nc.scalar.dma_start(wo_t[h][:], woT_r[:, h, :])

        # ---- Phase 2+3: attention with O-projection interleaved as PE filler ----
        ep = ctx.enter_context(tc.tile_pool(name="expp", bufs=3))
        np_pool = ctx.enter_context(tc.tile_pool(name="normp", bufs=2))
        osp = ctx.enter_context(tc.tile_pool(name="ostage", bufs=2))

        with tc.tile_pool(name="apss", bufs=3, space="PSUM") as pss_pool, \
             tc.tile_pool(name="aacc", bufs=4, space="PSUM") as acc_pool, \
             tc.tile_pool(name="apsd", bufs=1, space="PSUM") as psd_pool:

            orows = {}      # st -> staged output row awaiting DMA
            fillers = []    # pending O-proj (st, eo) units for PE filler

            def emit_filler(n):
                for _ in range(n):
                    if not fillers:
                        return
                    st, eo = fillers.pop(0)
                    pso = acc_pool.tile([P, SCH], F32, tag="acc", name=f"pso_{st}_{eo}")
                    for h in range(HPC):
                        nc.tensor.matmul(
                            pso[:],
                            attn_sb[:, h, st * P:(st + 1) * P],
                            wo_t[h][:, eo * SCH:(eo + 1) * SCH],
                            start=(h == 0), stop=(h == HPC - 1))
                    orow = orows[st]
                    nc.vector.tensor_copy(orow[:, eo * SCH:(eo + 1) * SCH], pso[:])
                    if eo == NEO - 1:
                        nc.sync.dma_start(outd[st * P:(st + 1) * P, :], orow[:])
                        del orows[st]

            def queue_oproj(c):
                for st in range(4 * c, 4 * c + 4):
                    orows[st] = osp.tile([P, E], F16, tag="orow", name=f"orow{st}")
                    for eo in range(NEO):
                        fillers.append((st, eo))

            for ai, c in enumerate(AT_ORDER):
                blocks = _allowed_tiles(c)
                nblk = len(blocks)
                for h in range(HPC):
                    psa = acc_pool.tile([P, SCH], F32, tag="acc", name=f"psa{h}")
                    psd = psd_pool.tile([1, SCH], F32, tag="psd")
                    # software pipeline: scores one block ahead of exp/AV
                    pend = None
                    for idx, (bj, lo, hi, mask, mpos) in enumerate(blocks):
                        n = (hi - lo) * P
                        pss = pss_pool.tile([P, SCH], F32, tag="pss")
                        nc.tensor.matmul(
                            pss[:, :n],
                            kT_sb[:, bj * P:(bj + 1) * P],
                            qT_sb[:, h, c * SCH + lo * P: c * SCH + hi * P],
                            start=True, stop=True)
                        et = ep.tile([P, SCH], BF16, tag="exp")
                        nc.scalar.activation(
                            et[:, lo * P:hi * P], pss[:, :n],
                            mybir.ActivationFunctionType.Exp, scale=SCALE)
                        if mask == "low":
                            nc.vector.tensor_tensor(
                                et[:, mpos * P:(mpos + 1) * P],
                                et[:, mpos * P:(mpos + 1) * P],
                                ml_sb[:], mybir.AluOpType.mult)
                        elif mask == "up":
                            nc.vector.tensor_tensor(
                                et[:, mpos * P:(mpos + 1) * P],
                                et[:, mpos * P:(mpos + 1) * P],
                                mu_sb[:], mybir.AluOpType.mult)
                        if pend is not None:
                            _emit_av(nc, psa, psd, v_sb, ones_sb, pend, nblk)
                            emit_filler(1)
                        pend = (idx, bj, lo, hi, et)
                    _emit_av(nc, psa, psd, v_sb, ones_sb, pend, nblk)
                    emit_filler(1)
                    # drain denominator via ScalarE (frees psd bank fast),
                    # then rc = ~1/denom on DVE, broadcast, multiply.
                    dsb = np_pool.tile([1, SCH], F32, tag="dsb")
                    nc.scalar.copy(dsb[:], psd[:])
                    rc = np_pool.tile([1, SCH], F32, tag="recip")
                    nc.vector.reciprocal_approx_fast(rc[:], dsb[:])
                    bc = np_pool.tile([P, SCH], F32, tag="bcast")
                    nc.gpsimd.partition_broadcast(bc[:], rc[:])
                    nc.vector.tensor_tensor(
                        attn_sb[:, h, c * SCH:(c + 1) * SCH], psa[:], bc[:],
                        mybir.AluOpType.mult)
                    emit_filler(2)
                if ai > 0:
                    emit_filler(len(fillers))  # drain any leftovers
                queue_oproj(c)
            emit_filler(len(fillers))
    nc.compile()
    return nc


def _emit_av(nc, psa, psd, v_sb, ones_sb, pend, nblk):
    idx, bj, lo, hi, et = pend
    sl = slice(lo * P, hi * P)
    nc.tensor.matmul(
        psa[:, sl], v_sb[:, bj, :], et[:, sl],
        start=(idx == 0), stop=(idx == nblk - 1))
    nc.tensor.matmul(
        psd[:, sl], ones_sb[:], et[:, sl],
        start=(idx == 0), stop=(idx == nblk - 1))


_NC_CACHE = {}


def get_nc():
    if "nc" not in _NC_CACHE:
        _NC_CACHE["nc"] = build_nc()
    return _NC_CACHE["nc"]


def make_in_maps(hidden_states, Wq, Wk, Wv, Wo):
    hid = np.asarray(hidden_states).reshape(S, E)
    hidT16 = np.ascontiguousarray(hid.T).astype(np.float16)

    inv = 1.0 / (10000.0 ** (np.arange(0, D, 2, dtype=np.float64) / D))
    t = np.arange(S, dtype=np.float64)
    fr = np.outer(t, inv)                      # [S, 64]
    emb = np.concatenate([fr, fr], axis=1)     # [S, 128]
    cosT = np.ascontiguousarray(np.cos(emb).T).astype(np.float32)
    sinF = np.ascontiguousarray(np.sin(emb).T).astype(np.float32)
    sinF[:64] *= -1.0                          # rotate_half sign fold
    sin2T = np.ascontiguousarray(np.roll(sinF, -64, axis=0))

    jj = np.arange(P)[:, None]
    ii = np.arange(P)[None, :]
    mlow = (jj >= ii).astype(ml_dtypes.bfloat16)   # block bj-bi=8: j-i>=1024
    mup = (ii >= jj).astype(ml_dtypes.bfloat16)    # block bi-bj=8: i-j>=1024

    in_maps = []
    for c in range(8):
        qsl = slice(c * 512, (c + 1) * 512)
        ksl = slice(c * 128, (c + 1) * 128)
        wqkv = np.concatenate(
            [Wq[qsl].T, Wk[ksl].T, Wv[ksl].T], axis=1)  # [E, 768]
        in_maps.append({
            "hidT": hidT16,
            "wqkvT": np.ascontiguousarray(wqkv).astype(np.float16),
            "woT": np.ascontiguousarray(Wo[:, qsl].T).astype(np.float16),
            "cosT": cosT,
            "sin2T": sin2T,
            "mlow": mlow,
            "mup": mup,
        })
    return in_maps


def run(in_maps, **kwargs):
    nc = get_nc()
    return run_bass_kernel_spmd(nc, in_maps, core_ids=list(range(8)), **kwargs)


def kernel(hidden_states, Wq, Wk, Wv, Wo):
    in_maps = make_in_maps(hidden_states, Wq, Wk, Wv, Wo)
    res = run(in_maps)
    out = np.zeros((S, E), dtype=np.float32)
    for r in res.results:
        out += r["out"].astype(np.float32)
    return out.reshape(1, S, E)


# revision 8
# speedup vs baseline: 1.4749x; 1.0839x over previous
"""Trainium2 Bass kernel for Mistral-style attention with an INVERTED band mask.

Reference semantics (S=2048, E=4096, H=32, KV=8, D=128, WINDOW=1024):
  q/k/v projections -> RoPE(q,k) -> GQA attention where positions with
  |i-j| < 1024 are masked OUT (attend only to far positions) -> softmax ->
  out projection.

Sharding (8 cores, tensor-parallel by GQA group):
  core c owns KV head c and Q heads 4c..4c+3. Column-parallel QKV,
  row-parallel O projection; the 8 fp16 partial outputs are summed on host.

On-device layout: everything transposed so matmuls contract on partitions.
  Host passes hidden^T, fused Wqkv^T slice, Wo^T slice, RoPE tables
  (transposed, sign-folded; sin2 = sin rolled by 64 so the rotate-half can
  happen AFTER the multiply via one SBUF->SBUF DMA swap), and two 128x128
  triangular masks for the blocks straddling the |i-j|=1024 boundary.

Block sparsity: score block (bi,bj) [128x128] is computed only when
  |bi-bj| >= 8; blocks at exactly |bi-bj|=8 get a triangular mask.

Scheduling notes (v3):
  - Two HWDGE rings: Sync carries latency-critical streams (hid tiles JIT
    slot-gated, output rows); Scalar carries bulk weights + rope swaps +
    V transposes, so a slot-wait on one stream can't head-of-line block
    the other.
  - Phase-1 chunk order [0,1,3,2] and attention order [3,2,0,1] so the
    first attention chunk's inputs are ready before phase 1 finishes.
  - O-projection matmuls of the previous attention chunk interleave into
    the block loop as PE filler while ScalarE runs exp; scores pipelined
    one block ahead (pss bufs=3); psa/pso share one 4-buf PSUM tag.
  - AV / denominator matmuls restricted to the valid query range; psd
    drained by a ScalarE copy so its bank frees independent of the DVE
    queue; reciprocal via reciprocal_approx_fast.
"""

import math
from contextlib import ExitStack

import numpy as np
import ml_dtypes

import concourse.bass as bass
import concourse.mybir as mybir
import concourse.tile as tile
from concourse import bacc
from concourse.bass_utils import run_bass_kernel_spmd

P = 128
S = 2048
E = 4096
D = 128
HPC = 4          # q heads per core
NE = E // P      # 32 e-tiles
NE2 = NE // 2    # 16 double-e tiles
NSCH = 4         # s-chunks of 512
SCH = S // NSCH  # 512
NST = S // P     # 16 s-tiles
NEO = 8          # output e-chunks of 512
WQKV = HPC * D + 2 * D  # 768 fused qkv weight cols per e-tile
SCALE = 1.0 / math.sqrt(D)
F16 = mybir.dt.float16
F32 = mybir.dt.float32
BF16 = mybir.dt.bfloat16

P1_ORDER = [0, 1, 2, 3]   # phase-1 chunk order
AT_ORDER = [2, 3, 0, 1]   # attention chunk order
HID_AHEAD = 6             # hid DMA lookahead (in [128,2,512] tiles)


def _allowed_tiles(c):
    """For s-chunk c (query blocks bi=4c..4c+3), list (bj, lo, hi, mask, mpos):
    key tile bj is needed for query sub-tiles [lo, hi) (chunk-relative);
    mask in {None,'low','up'} applied at chunk-relative position mpos."""
    out = []
    bis = range(4 * c, 4 * c + 4)
    for bj in range(NST):
        ok = [bi for bi in bis if abs(bi - bj) >= 8]
        if not ok:
            continue
        lo = min(ok) - 4 * c
        hi = max(ok) + 1 - 4 * c
        assert ok == list(range(lo + 4 * c, hi + 4 * c)), (c, bj, ok)
        mask, mpos = None, 0
        if bj - 8 in ok:
            mask, mpos = "low", bj - 8 - 4 * c
        elif bj + 8 in ok:
            mask, mpos = "up", bj + 8 - 4 * c
        out.append((bj, lo, hi, mask, mpos))
    return out


def build_nc(debug=False):
    nc = bacc.Bacc("TRN2", target_bir_lowering=False, debug=False)
    hidT = nc.dram_tensor("hidT", (E, S), F16, kind="ExternalInput")
    wqkvT = nc.dram_tensor("wqkvT", (E, WQKV), F16, kind="ExternalInput")
    woT = nc.dram_tensor("woT", (HPC * D, E), F16, kind="ExternalInput")
    cosT = nc.dram_tensor("cosT", (D, S), F32, kind="ExternalInput")
    sin2T = nc.dram_tensor("sin2T", (D, S), F32, kind="ExternalInput")
    mlow = nc.dram_tensor("mlow", (P, P), BF16, kind="ExternalInput")
    mup = nc.dram_tensor("mup", (P, P), BF16, kind="ExternalInput")
    outd = nc.dram_tensor("out", (S, E), F16, kind="ExternalOutput")

    with tile.TileContext(nc) as tc, ExitStack() as ctx:
        const = ctx.enter_context(tc.tile_pool(name="const", bufs=1))

        wqkvT_r = wqkvT.rearrange("(eo p) d -> p eo d", p=P)
        woT_r = woT.rearrange("(ho p) e -> p ho e", p=P)
        hidT_r = hidT.rearrange("(eo p) s -> p eo s", p=P)

        # SBUF homes (persistent)
        qT_sb = const.tile([P, HPC, S], F16)     # Q^T per head [d, s]
        kT_sb = const.tile([P, S], F16)          # K^T [d, s]
        v_sb = const.tile([P, NST, D], F16)      # V [s-tile, d]
        attn_sb = const.tile([P, HPC, S], F16)   # attn_out^T per head [d, s]

        wqkv_t = [const.tile([P, WQKV], F16, name=f"wqkv{e}") for e in range(NE)]
        wo_t = [const.tile([P, E], F16, name=f"wo{h}") for h in range(HPC)]
        cos_sb = const.tile([P, S], F32)
        sin2_sb = const.tile([P, S], F32)
        ml_sb = const.tile([P, P], BF16)
        mu_sb = const.tile([P, P], BF16)
        ones_sb = const.tile([P, P], F16)

        def wq_ap(e, h):
            return wqkv_t[e][:, h * D:(h + 1) * D]

        def wk_ap(e):
            return wqkv_t[e][:, HPC * D:HPC * D + D]

        def wv_ap(e):
            return wqkv_t[e][:, HPC * D + D:]

        hidp = ctx.enter_context(tc.tile_pool(name="hid", bufs=HID_AHEAD))
        hid_tiles = {}

        def issue_hid_dma(c, e2):
            ht = hidp.tile([P, 2, SCH], F16, tag="hid")
            nc.sync.dma_start(
                ht[:], hidT_r[:, 2 * e2:2 * e2 + 2, c * SCH:(c + 1) * SCH])
            hid_tiles[(c, e2)] = ht

        # --- upfront DMA program (scalar ring for weights; sync for hid) ---
        c0 = P1_ORDER[0]
        for j in range(HID_AHEAD):
            issue_hid_dma(c0, j)
            nc.scalar.dma_start(wqkv_t[2 * j][:], wqkvT_r[:, 2 * j, :])
            nc.scalar.dma_start(wqkv_t[2 * j + 1][:], wqkvT_r[:, 2 * j + 1, :])
            if j == 0:
                nc.gpsimd.memset(ones_sb[:], 1.0)
            if j == 2:
                nc.scalar.dma_start(cos_sb[:], cosT[:])
                nc.scalar.dma_start(sin2_sb[:], sin2T[:])
            if j == 4:
                nc.scalar.dma_start(ml_sb[:], mlow[:])
                nc.scalar.dma_start(mu_sb[:], mup[:])
        for e in range(2 * HID_AHEAD, NE):
            nc.scalar.dma_start(wqkv_t[e][:], wqkvT_r[:, e, :])

        rp = ctx.enter_context(tc.tile_pool(name="rope", bufs=2))

        def rope_chunk(tens, c):
            """For each (src_psum, dst): dst = src*cos + rot64(src*sin2).
            All multiplies emitted first so PSUM banks release at DVE line
            rate; rot swaps (scalar-ring SBUF DMAs) land while the DVE
            works; the in-place adds then never block the DVE FIFO."""
            csl = slice(c * SCH, (c + 1) * SCH)
            t2s = []
            for i, (ps, dst) in enumerate(tens):
                nc.vector.tensor_tensor(dst, ps, cos_sb[:, csl],
                                        mybir.AluOpType.mult)
                t2 = rp.tile([P, SCH], F16, tag="t2", bufs=5, name=f"t2_{i}")
                nc.vector.tensor_tensor(t2[:], ps, sin2_sb[:, csl],
                                        mybir.AluOpType.mult)
                t2s.append(t2)
            rots = []
            for i, t2 in enumerate(t2s):
                rot = rp.tile([P, SCH], F16, tag="rot", bufs=5, name=f"rot_{i}")
                nc.scalar.dma_start(rot[0:64, :], t2[64:128, :])
                nc.scalar.dma_start(rot[64:128, :], t2[0:64, :])
                rots.append(rot)
            for (ps, dst), rot in zip(tens, rots):
                nc.vector.tensor_tensor(dst, dst, rot[:], mybir.AluOpType.add)

        # ---- Phase 1: QKV projections (+RoPE) ----
        with tc.tile_pool(name="p1q", bufs=6, space="PSUM") as p1q, \
             tc.tile_pool(name="p1k", bufs=1, space="PSUM") as p1k, \
             tc.tile_pool(name="p1v", bufs=1, space="PSUM") as p1v:
            for ci, c in enumerate(P1_ORDER):
                psq = [p1q.tile([P, SCH], F32, tag="psq", name=f"psq{h}")
                       for h in range(HPC)]
                psk = p1k.tile([P, SCH], F32, tag="psk")
                psvT = p1v.tile([P, SCH], F32, tag="psv")
                for e2 in range(NE2):
                    ht = hid_tiles.pop((c, e2))
                    haps = [ht[:, j, :] for j in range(2)]
                    flags = [((2 * e2 + j) == 0, (2 * e2 + j) == NE - 1)
                             for j in range(2)]
                    # k/v first (lead-in work while psq banks free up)
                    for j in range(2):
                        e = 2 * e2 + j
                        st, sp = flags[j]
                        nc.tensor.matmul(psk[:], wk_ap(e), haps[j],
                                         start=st, stop=sp)
                        nc.tensor.matmul(psvT[:], wv_ap(e), haps[j],
                                         start=st, stop=sp)
                    for j in range(2):
                        e = 2 * e2 + j
                        st, sp = flags[j]
                        for h in range(HPC):
                            nc.tensor.matmul(psq[h][:], wq_ap(e, h), haps[j],
                                             start=st, stop=sp)
                    # JIT prefetch with HID_AHEAD tiles of lookahead
                    nxt = e2 + HID_AHEAD
                    if nxt < NE2:
                        issue_hid_dma(c, nxt)
                    elif ci + 1 < NSCH:
                        issue_hid_dma(P1_ORDER[ci + 1], nxt - NE2)
                vstage = rp.tile([P, SCH], F16, tag="vstage", bufs=2)
                nc.scalar.copy(vstage[:], psvT[:])
                nc.scalar.dma_start_transpose(
                    v_sb[:, c * 4:(c + 1) * 4, :], vstage[:])
                tens = [(psk[:], kT_sb[:, c * SCH:(c + 1) * SCH])]
                tens += [(psq[h][:], qT_sb[:, h, c * SCH:(c + 1) * SCH])
                         for h in range(HPC)]
                rope_chunk(tens, c)
                if ci == 1:
                    # bulk wo loads: needed first ~40us into attention
                    for h in range(HPC):
                        nc.scalar.dma_start(wo_t[h][:], woT_r[:, h, :])

        # ---- Phase 2+3: attention with O-projection interleaved as PE filler ----
        ep = ctx.enter_context(tc.tile_pool(name="expp", bufs=3))
        np_pool = ctx.enter_context(tc.tile_pool(name="normp", bufs=2))
        osp = ctx.enter_context(tc.tile_pool(name="ostage", bufs=2))

        with tc.tile_pool(name="apss", bufs=3, space="PSUM") as pss_pool, \
             tc.tile_pool(name="aacc", bufs=4, space="PSUM") as acc_pool, \
             tc.tile_pool(name="apsd", bufs=1, space="PSUM") as psd_pool:

            orows = {}      # st -> staged output row awaiting DMA
            fillers = []    # pending O-proj (st, eo) units for PE filler

            def emit_filler(n):
                for _ in range(n):
                    if not fillers:
                        return
                    st, eo = fillers.pop(0)
                    pso = acc_pool.tile([P, SCH], F32, tag="acc", name=f"pso_{st}_{eo}")
                    for h in range(HPC):
                        nc.tensor.matmul(
                            pso[:],
                            attn_sb[:, h, st * P:(st + 1) * P],
                            wo_t[h][:, eo * SCH:(eo + 1) * SCH],
                            start=(h == 0), stop=(h == HPC - 1))
                    orow = orows[st]
                    nc.vector.tensor_copy(orow[:, eo * SCH:(eo + 1) * SCH], pso[:])
                    if eo == NEO // 2 - 1:
                        nc.sync.dma_start(
                            outd[st * P:(st + 1) * P, :E // 2],
                            orow[:, :E // 2])
                    elif eo == NEO - 1:
                        nc.sync.dma_start(
                            outd[st * P:(st + 1) * P, E // 2:],
                            orow[:, E // 2:])
                        del orows[st]

            def queue_oproj(c):
                for st in range(4 * c, 4 * c + 4):
                    orows[st] = osp.tile([P, E], F16, tag="orow", name=f"orow{st}")
                    for eo in range(NEO):
                        fillers.append((st, eo))

            for ai, c in enumerate(AT_ORDER):
                blocks = _allowed_tiles(c)
                nblk = len(blocks)
                for h in range(HPC):
                    psa = acc_pool.tile([P, SCH], F32, tag="acc", name=f"psa{h}")
                    psd = psd_pool.tile([P, SCH], F32, tag="psd")
                    # software pipeline: scores one block ahead of exp/AV
                    pend = None
                    for idx, (bj, lo, hi, mask, mpos) in enumerate(blocks):
                        n = (hi - lo) * P
                        pss = pss_pool.tile([P, SCH], F32, tag="pss")
                        nc.tensor.matmul(
                            pss[:, :n],
                            kT_sb[:, bj * P:(bj + 1) * P],
                            qT_sb[:, h, c * SCH + lo * P: c * SCH + hi * P],
                            start=True, stop=True)
                        et = ep.tile([P, SCH], BF16, tag="exp")
                        nc.scalar.activation(
                            et[:, lo * P:hi * P], pss[:, :n],
                            mybir.ActivationFunctionType.Exp, scale=SCALE)
                        if mask == "low":
                            nc.vector.tensor_tensor(
                                et[:, mpos * P:(mpos + 1) * P],
                                et[:, mpos * P:(mpos + 1) * P],
                                ml_sb[:], mybir.AluOpType.mult)
                        elif mask == "up":
                            nc.vector.tensor_tensor(
                                et[:, mpos * P:(mpos + 1) * P],
                                et[:, mpos * P:(mpos + 1) * P],
                                mu_sb[:], mybir.AluOpType.mult)
                        if pend is not None:
                            _emit_av(nc, psa, psd, v_sb, ones_sb, pend, nblk)
                            emit_filler(1)
                        pend = (idx, bj, lo, hi, et)
                    _emit_av(nc, psa, psd, v_sb, ones_sb, pend, nblk)
                    emit_filler(1)
                    # drain denominator via ScalarE (frees psd bank fast),
                    # then rc = ~1/denom on DVE, broadcast, multiply.
                    dsb = np_pool.tile([1, SCH], F32, tag="dsb")
                    nc.scalar.copy(dsb[:], psd[0:1, :])
                    rc = np_pool.tile([1, SCH], F32, tag="recip")
                    nc.vector.reciprocal_approx_fast(rc[:], dsb[:])
                    bc = np_pool.tile([P, SCH], F32, tag="bcast")
                    nc.gpsimd.partition_broadcast(bc[:], rc[:])
                    nc.vector.tensor_tensor(
                        attn_sb[:, h, c * SCH:(c + 1) * SCH], psa[:], bc[:],
                        mybir.AluOpType.mult)
                    emit_filler(2)
                if ai > 0:
                    emit_filler(len(fillers))  # drain any leftovers
                queue_oproj(c)
            emit_filler(len(fillers))
    nc.compile()
    return nc


def _emit_av(nc, psa, psd, v_sb, ones_sb, pend, nblk):
    idx, bj, lo, hi, et = pend
    sl = slice(lo * P, hi * P)
    nc.tensor.matmul(
        psa[:, sl], v_sb[:, bj, :], et[:, sl],
        start=(idx == 0), stop=(idx == nblk - 1))
    nc.tensor.matmul(
        psd[:, sl], ones_sb[:], et[:, sl],
        start=(idx == 0), stop=(idx == nblk - 1))


_NC_CACHE = {}


def get_nc():
    if "nc" not in _NC_CACHE:
        _NC_CACHE["nc"] = build_nc()
    return _NC_CACHE["nc"]


def make_in_maps(hidden_states, Wq, Wk, Wv, Wo):
    hid = np.asarray(hidden_states).reshape(S, E)
    hidT16 = np.ascontiguousarray(hid.T).astype(np.float16)

    inv = 1.0 / (10000.0 ** (np.arange(0, D, 2, dtype=np.float64) / D))
    t = np.arange(S, dtype=np.float64)
    fr = np.outer(t, inv)                      # [S, 64]
    emb = np.concatenate([fr, fr], axis=1)     # [S, 128]
    cosT = np.ascontiguousarray(np.cos(emb).T).astype(np.float32)
    sinF = np.ascontiguousarray(np.sin(emb).T).astype(np.float32)
    sinF[:64] *= -1.0                          # rotate_half sign fold
    sin2T = np.ascontiguousarray(np.roll(sinF, -64, axis=0))

    jj = np.arange(P)[:, None]
    ii = np.arange(P)[None, :]
    mlow = (jj >= ii).astype(ml_dtypes.bfloat16)   # block bj-bi=8: j-i>=1024
    mup = (ii >= jj).astype(ml_dtypes.bfloat16)    # block bi-bj=8: i-j>=1024

    in_maps = []
    for c in range(8):
        qsl = slice(c * 512, (c + 1) * 512)
        ksl = slice(c * 128, (c + 1) * 128)
        wqkv = np.concatenate(
            [Wq[qsl].T, Wk[ksl].T, Wv[ksl].T], axis=1)  # [E, 768]
        in_maps.append({
            "hidT": hidT16,
            "wqkvT": np.ascontiguousarray(wqkv).astype(np.float16),
            "woT": np.ascontiguousarray(Wo[:, qsl].T).astype(np.float16),
            "cosT": cosT,
            "sin2T": sin2T,
            "mlow": mlow,
            "mup": mup,
        })
    return in_maps


def run(in_maps, **kwargs):
    nc = get_nc()
    return run_bass_kernel_spmd(nc, in_maps, core_ids=list(range(8)), **kwargs)


def kernel(hidden_states, Wq, Wk, Wv, Wo):
    in_maps = make_in_maps(hidden_states, Wq, Wk, Wv, Wo)
    res = run(in_maps)
    out = np.zeros((S, E), dtype=np.float32)
    for r in res.results:
        out += r["out"].astype(np.float32)
    return out.reshape(1, S, E)


# revision 9
# speedup vs baseline: 1.5368x; 1.0419x over previous
"""Trainium2 Bass kernel for Mistral-style attention with an INVERTED band mask.

Reference semantics (S=2048, E=4096, H=32, KV=8, D=128, WINDOW=1024):
  q/k/v projections -> RoPE(q,k) -> GQA attention where positions with
  |i-j| < 1024 are masked OUT (attend only to far positions) -> softmax ->
  out projection.

Sharding (8 cores, tensor-parallel by GQA group):
  core c owns KV head c and Q heads 4c..4c+3. Column-parallel QKV,
  row-parallel O projection; the 8 fp16 partial outputs are summed on host.

On-device layout: everything transposed so matmuls contract on partitions.
  Host passes hidden^T, fused Wqkv^T slice, Wo^T slice, RoPE tables
  (transposed, sign-folded; sin2 = sin rolled by 64 so the rotate-half can
  happen AFTER the multiply via one SBUF->SBUF DMA swap), and two 128x128
  triangular masks for the blocks straddling the |i-j|=1024 boundary.

Block sparsity: score block (bi,bj) [128x128] is computed only when
  |bi-bj| >= 8; blocks at exactly |bi-bj|=8 get a triangular mask.

Scheduling notes (v3):
  - Two HWDGE rings: Sync carries latency-critical streams (hid tiles JIT
    slot-gated, output rows); Scalar carries bulk weights + rope swaps +
    V transposes, so a slot-wait on one stream can't head-of-line block
    the other.
  - Phase-1 chunk order [0,1,3,2] and attention order [3,2,0,1] so the
    first attention chunk's inputs are ready before phase 1 finishes.
  - O-projection matmuls of the previous attention chunk interleave into
    the block loop as PE filler while ScalarE runs exp; scores pipelined
    one block ahead (pss bufs=3); psa/pso share one 4-buf PSUM tag.
  - AV / denominator matmuls restricted to the valid query range; psd
    drained by a ScalarE copy so its bank frees independent of the DVE
    queue; reciprocal via reciprocal_approx_fast.
"""

import math
from contextlib import ExitStack

import numpy as np
import ml_dtypes

import concourse.bass as bass
import concourse.mybir as mybir
import concourse.tile as tile
from concourse import bacc
from concourse.bass_utils import run_bass_kernel_spmd

P = 128
S = 2048
E = 4096
D = 128
HPC = 4          # q heads per core
NE = E // P      # 32 e-tiles
NE2 = NE // 2    # 16 double-e tiles
NSCH = 4         # s-chunks of 512
SCH = S // NSCH  # 512
NST = S // P     # 16 s-tiles
NEO = 8          # output e-chunks of 512
WQKV = HPC * D + 2 * D  # 768 fused qkv weight cols per e-tile
SCALE = 1.0 / math.sqrt(D)
F16 = mybir.dt.float16
F32 = mybir.dt.float32
BF16 = mybir.dt.bfloat16

P1_ORDER = [0, 1, 2, 3]   # phase-1 chunk order
AT_ORDER = [2, 3, 0, 1]   # attention chunk order
HID_AHEAD = 8             # hid DMA lookahead (in [128,2,512] tiles)


def _allowed_tiles(c):
    """For s-chunk c (query blocks bi=4c..4c+3), list (bj, lo, hi, mask, mpos):
    key tile bj is needed for query sub-tiles [lo, hi) (chunk-relative);
    mask in {None,'low','up'} applied at chunk-relative position mpos."""
    out = []
    bis = range(4 * c, 4 * c + 4)
    for bj in range(NST):
        ok = [bi for bi in bis if abs(bi - bj) >= 8]
        if not ok:
            continue
        lo = min(ok) - 4 * c
        hi = max(ok) + 1 - 4 * c
        assert ok == list(range(lo + 4 * c, hi + 4 * c)), (c, bj, ok)
        mask, mpos = None, 0
        if bj - 8 in ok:
            mask, mpos = "low", bj - 8 - 4 * c
        elif bj + 8 in ok:
            mask, mpos = "up", bj + 8 - 4 * c
        out.append((bj, lo, hi, mask, mpos))
    return out


def build_nc(debug=False):
    nc = bacc.Bacc("TRN2", target_bir_lowering=False, debug=False)
    hidT = nc.dram_tensor("hidT", (E, S), F16, kind="ExternalInput")
    wqkvT = nc.dram_tensor("wqkvT", (E, WQKV), F16, kind="ExternalInput")
    woT = nc.dram_tensor("woT", (HPC * D, E), F16, kind="ExternalInput")
    cosT = nc.dram_tensor("cosT", (D, S), F32, kind="ExternalInput")
    sin2T = nc.dram_tensor("sin2T", (D, S), F32, kind="ExternalInput")
    mlow = nc.dram_tensor("mlow", (P, P), BF16, kind="ExternalInput")
    mup = nc.dram_tensor("mup", (P, P), BF16, kind="ExternalInput")
    outd = nc.dram_tensor("out", (S, E), F16, kind="ExternalOutput")

    with tile.TileContext(nc) as tc, ExitStack() as ctx:
        const = ctx.enter_context(tc.tile_pool(name="const", bufs=1))

        wqkvT_r = wqkvT.rearrange("(eo p) d -> p eo d", p=P)
        woT_r = woT.rearrange("(ho p) e -> p ho e", p=P)
        hidT_r = hidT.rearrange("(eo p) s -> p eo s", p=P)

        # SBUF homes (persistent)
        qT_sb = const.tile([P, HPC, S], F16)     # Q^T per head [d, s]
        kT_sb = const.tile([P, S], F16)          # K^T [d, s]
        v_sb = const.tile([P, NST, D], F16)      # V [s-tile, d]
        attn_sb = const.tile([P, HPC, S], F16)   # attn_out^T per head [d, s]

        wqkv_t = [const.tile([P, WQKV], F16, name=f"wqkv{e}") for e in range(NE)]
        wo_t = [const.tile([P, E], F16, name=f"wo{h}") for h in range(HPC)]
        cos_sb = const.tile([P, S], F32)
        sin2_sb = const.tile([P, S], F32)
        ml_sb = const.tile([P, P], BF16)
        mu_sb = const.tile([P, P], BF16)
        ones_sb = const.tile([P, P], F16)

        def wq_ap(e, h):
            return wqkv_t[e][:, h * D:(h + 1) * D]

        def wk_ap(e):
            return wqkv_t[e][:, HPC * D:HPC * D + D]

        def wv_ap(e):
            return wqkv_t[e][:, HPC * D + D:]

        hidp = ctx.enter_context(tc.tile_pool(name="hid", bufs=HID_AHEAD))
        hid_tiles = {}

        def issue_hid_dma(c, e2):
            ht = hidp.tile([P, 2, SCH], F16, tag="hid")
            nc.sync.dma_start(
                ht[:], hidT_r[:, 2 * e2:2 * e2 + 2, c * SCH:(c + 1) * SCH])
            hid_tiles[(c, e2)] = ht

        # --- upfront DMA program (scalar ring for weights; sync for hid) ---
        c0 = P1_ORDER[0]
        for j in range(HID_AHEAD):
            issue_hid_dma(c0, j)
            nc.scalar.dma_start(wqkv_t[2 * j][:], wqkvT_r[:, 2 * j, :])
            nc.scalar.dma_start(wqkv_t[2 * j + 1][:], wqkvT_r[:, 2 * j + 1, :])
            if j == 0:
                nc.gpsimd.memset(ones_sb[:], 1.0)
        for e in range(2 * HID_AHEAD, NE):
            nc.scalar.dma_start(wqkv_t[e][:], wqkvT_r[:, e, :])
        # rope tables / masks: first needed at the first chunk's rope (~50us in)
        nc.scalar.dma_start(cos_sb[:], cosT[:])
        nc.scalar.dma_start(sin2_sb[:], sin2T[:])
        nc.scalar.dma_start(ml_sb[:], mlow[:])
        nc.scalar.dma_start(mu_sb[:], mup[:])

        rp = ctx.enter_context(tc.tile_pool(name="rope", bufs=2))

        def rope_chunk(tens, c):
            """For each (src_psum, dst): dst = src*cos + rot64(src*sin2).
            All multiplies emitted first so PSUM banks release at DVE line
            rate; rot swaps (scalar-ring SBUF DMAs) land while the DVE
            works; the in-place adds then never block the DVE FIFO."""
            csl = slice(c * SCH, (c + 1) * SCH)
            t2s = []
            for i, (ps, dst) in enumerate(tens):
                nc.vector.tensor_tensor(dst, ps, cos_sb[:, csl],
                                        mybir.AluOpType.mult)
                t2 = rp.tile([P, SCH], F16, tag="t2", bufs=5, name=f"t2_{i}")
                nc.vector.tensor_tensor(t2[:], ps, sin2_sb[:, csl],
                                        mybir.AluOpType.mult)
                t2s.append(t2)
            rots = []
            for i, t2 in enumerate(t2s):
                rot = rp.tile([P, SCH], F16, tag="rot", bufs=5, name=f"rot_{i}")
                nc.scalar.dma_start(rot[0:64, :], t2[64:128, :])
                nc.scalar.dma_start(rot[64:128, :], t2[0:64, :])
                rots.append(rot)
            for (ps, dst), rot in zip(tens, rots):
                nc.vector.tensor_tensor(dst, dst, rot[:], mybir.AluOpType.add)

        # ---- Phase 1: QKV projections (+RoPE) ----
        with tc.tile_pool(name="p1q", bufs=6, space="PSUM") as p1q, \
             tc.tile_pool(name="p1k", bufs=1, space="PSUM") as p1k, \
             tc.tile_pool(name="p1v", bufs=1, space="PSUM") as p1v:
            for ci, c in enumerate(P1_ORDER):
                psq = [p1q.tile([P, SCH], F32, tag="psq", name=f"psq{h}")
                       for h in range(HPC)]
                psk = p1k.tile([P, SCH], F32, tag="psk")
                psvT = p1v.tile([P, SCH], F32, tag="psv")
                for e2 in range(NE2):
                    ht = hid_tiles.pop((c, e2))
                    haps = [ht[:, j, :] for j in range(2)]
                    flags = [((2 * e2 + j) == 0, (2 * e2 + j) == NE - 1)
                             for j in range(2)]
                    # k/v first (lead-in work while psq banks free up)
                    for j in range(2):
                        e = 2 * e2 + j
                        st, sp = flags[j]
                        nc.tensor.matmul(psk[:], wk_ap(e), haps[j],
                                         start=st, stop=sp)
                        nc.tensor.matmul(psvT[:], wv_ap(e), haps[j],
                                         start=st, stop=sp)
                    for j in range(2):
                        e = 2 * e2 + j
                        st, sp = flags[j]
                        for h in range(HPC):
                            nc.tensor.matmul(psq[h][:], wq_ap(e, h), haps[j],
                                             start=st, stop=sp)
                    # JIT prefetch with HID_AHEAD tiles of lookahead
                    nxt = e2 + HID_AHEAD
                    if nxt < NE2:
                        issue_hid_dma(c, nxt)
                    elif ci + 1 < NSCH:
                        issue_hid_dma(P1_ORDER[ci + 1], nxt - NE2)
                vstage = rp.tile([P, SCH], F16, tag="vstage", bufs=2)
                nc.scalar.copy(vstage[:], psvT[:])
                nc.scalar.dma_start_transpose(
                    v_sb[:, c * 4:(c + 1) * 4, :], vstage[:])
                tens = [(psk[:], kT_sb[:, c * SCH:(c + 1) * SCH])]
                tens += [(psq[h][:], qT_sb[:, h, c * SCH:(c + 1) * SCH])
                         for h in range(HPC)]
                rope_chunk(tens, c)
                if ci == 1:
                    # bulk wo loads: needed first ~40us into attention
                    for h in range(HPC):
                        nc.scalar.dma_start(wo_t[h][:], woT_r[:, h, :])

        # ---- Phase 2+3: attention with O-projection interleaved as PE filler ----
        ep = ctx.enter_context(tc.tile_pool(name="expp", bufs=3))
        np_pool = ctx.enter_context(tc.tile_pool(name="normp", bufs=2))
        osp = ctx.enter_context(tc.tile_pool(name="ostage", bufs=2))

        with tc.tile_pool(name="apss", bufs=3, space="PSUM") as pss_pool, \
             tc.tile_pool(name="aacc", bufs=4, space="PSUM") as acc_pool, \
             tc.tile_pool(name="apsd", bufs=1, space="PSUM") as psd_pool:

            orows = {}      # st -> staged output row awaiting DMA
            fillers = []    # pending O-proj (st, eo) units for PE filler

            def emit_filler(n):
                for _ in range(n):
                    if not fillers:
                        return
                    st, eo = fillers.pop(0)
                    pso = acc_pool.tile([P, SCH], F32, tag="acc", name=f"pso_{st}_{eo}")
                    for h in range(HPC):
                        nc.tensor.matmul(
                            pso[:],
                            attn_sb[:, h, st * P:(st + 1) * P],
                            wo_t[h][:, eo * SCH:(eo + 1) * SCH],
                            start=(h == 0), stop=(h == HPC - 1))
                    half = eo // (NEO // 2)
                    if eo % (NEO // 2) == 0:
                        orows[st] = osp.tile([P, E // 2], F16, tag="orow",
                                             name=f"orow{st}_{half}")
                    orow = orows[st]
                    nc.vector.tensor_copy(
                        orow[:, (eo % (NEO // 2)) * SCH:
                             (eo % (NEO // 2) + 1) * SCH], pso[:])
                    if eo % (NEO // 2) == NEO // 2 - 1:
                        nc.sync.dma_start(
                            outd[st * P:(st + 1) * P,
                                 half * (E // 2):(half + 1) * (E // 2)],
                            orow[:])
                        del orows[st]

            def queue_oproj(c):
                for st in range(4 * c, 4 * c + 4):
                    for eo in range(NEO):
                        fillers.append((st, eo))

            for ai, c in enumerate(AT_ORDER):
                blocks = _allowed_tiles(c)
                nblk = len(blocks)
                for h in range(HPC):
                    psa = acc_pool.tile([P, SCH], F32, tag="acc", name=f"psa{h}")
                    psd = psd_pool.tile([P, SCH], F32, tag="psd")
                    # software pipeline: scores one block ahead of exp/AV
                    pend = None
                    for idx, (bj, lo, hi, mask, mpos) in enumerate(blocks):
                        n = (hi - lo) * P
                        pss = pss_pool.tile([P, SCH], F32, tag="pss")
                        nc.tensor.matmul(
                            pss[:, :n],
                            kT_sb[:, bj * P:(bj + 1) * P],
                            qT_sb[:, h, c * SCH + lo * P: c * SCH + hi * P],
                            start=True, stop=True)
                        et = ep.tile([P, SCH], BF16, tag="exp")
                        nc.scalar.activation(
                            et[:, lo * P:hi * P], pss[:, :n],
                            mybir.ActivationFunctionType.Exp, scale=SCALE)
                        if mask == "low":
                            nc.vector.tensor_tensor(
                                et[:, mpos * P:(mpos + 1) * P],
                                et[:, mpos * P:(mpos + 1) * P],
                                ml_sb[:], mybir.AluOpType.mult)
                        elif mask == "up":
                            nc.vector.tensor_tensor(
                                et[:, mpos * P:(mpos + 1) * P],
                                et[:, mpos * P:(mpos + 1) * P],
                                mu_sb[:], mybir.AluOpType.mult)
                        if pend is not None:
                            _emit_av(nc, psa, psd, v_sb, ones_sb, pend, nblk)
                            emit_filler(1)
                        pend = (idx, bj, lo, hi, et)
                    _emit_av(nc, psa, psd, v_sb, ones_sb, pend, nblk)
                    emit_filler(1)
                    # drain denominator via ScalarE (frees psd bank fast),
                    # then rc = ~1/denom on DVE, broadcast, multiply.
                    dsb = np_pool.tile([1, SCH], F32, tag="dsb")
                    nc.scalar.copy(dsb[:], psd[0:1, :])
                    rc = np_pool.tile([1, SCH], F32, tag="recip")
                    nc.vector.reciprocal_approx_fast(rc[:], dsb[:])
                    bc = np_pool.tile([P, SCH], F32, tag="bcast")
                    nc.gpsimd.partition_broadcast(bc[:], rc[:])
                    nc.vector.tensor_tensor(
                        attn_sb[:, h, c * SCH:(c + 1) * SCH], psa[:], bc[:],
                        mybir.AluOpType.mult)
                    emit_filler(2)
                if ai > 0:
                    emit_filler(len(fillers))  # drain any leftovers
                queue_oproj(c)
            emit_filler(len(fillers))
    nc.compile()
    return nc


def _emit_av(nc, psa, psd, v_sb, ones_sb, pend, nblk):
    idx, bj, lo, hi, et = pend
    sl = slice(lo * P, hi * P)
    nc.tensor.matmul(
        psa[:, sl], v_sb[:, bj, :], et[:, sl],
        start=(idx == 0), stop=(idx == nblk - 1))
    nc.tensor.matmul(
        psd[:, sl], ones_sb[:], et[:, sl],
        start=(idx == 0), stop=(idx == nblk - 1))


_NC_CACHE = {}


def get_nc():
    if "nc" not in _NC_CACHE:
        _NC_CACHE["nc"] = build_nc()
    return _NC_CACHE["nc"]


def make_in_maps(hidden_states, Wq, Wk, Wv, Wo):
    hid = np.asarray(hidden_states).reshape(S, E)
    hidT16 = np.ascontiguousarray(hid.T).astype(np.float16)

    inv = 1.0 / (10000.0 ** (np.arange(0, D, 2, dtype=np.float64) / D))
    t = np.arange(S, dtype=np.float64)
    fr = np.outer(t, inv)                      # [S, 64]
    emb = np.concatenate([fr, fr], axis=1)     # [S, 128]
    cosT = np.ascontiguousarray(np.cos(emb).T).astype(np.float32)
    sinF = np.ascontiguousarray(np.sin(emb).T).astype(np.float32)
    sinF[:64] *= -1.0                          # rotate_half sign fold
    sin2T = np.ascontiguousarray(np.roll(sinF, -64, axis=0))

    jj = np.arange(P)[:, None]
    ii = np.arange(P)[None, :]
    mlow = (jj >= ii).astype(ml_dtypes.bfloat16)   # block bj-bi=8: j-i>=1024
    mup = (ii >= jj).astype(ml_dtypes.bfloat16)    # block bi-bj=8: i-j>=1024

    in_maps = []
    for c in range(8):
        qsl = slice(c * 512, (c + 1) * 512)
        ksl = slice(c * 128, (c + 1) * 128)
        wqkv = np.concatenate(
            [Wq[qsl].T, Wk[ksl].T, Wv[ksl].T], axis=1)  # [E, 768]
        in_maps.append({
            "hidT": hidT16,
            "wqkvT": np.ascontiguousarray(wqkv).astype(np.float16),
            "woT": np.ascontiguousarray(Wo[:, qsl].T).astype(np.float16),
            "cosT": cosT,
            "sin2T": sin2T,
            "mlow": mlow,
            "mup": mup,
        })
    return in_maps


def run(in_maps, **kwargs):
    nc = get_nc()
    return run_bass_kernel_spmd(nc, in_maps, core_ids=list(range(8)), **kwargs)


def kernel(hidden_states, Wq, Wk, Wv, Wo):
    in_maps = make_in_maps(hidden_states, Wq, Wk, Wv, Wo)
    res = run(in_maps)
    out = np.zeros((S, E), dtype=np.float32)
    for r in res.results:
        out += r["out"].astype(np.float32)
    return out.reshape(1, S, E)
